# revision 3
# baseline (speedup 1.0000x reference)
"""Trainium2 Bass kernel for DecouplePreAggGraphConv (GNN message passing).

out[b,j,:] = diag(adj)[j] * (x[b,j] @ W0[j])
           + sum_k offdiag(adj)[j,k] * (x[b,k] @ W1[k])
           + bias

Data-parallel over B across 8 NeuronCores. Low-FLOP ("smart") algorithm:
per-joint GEMMs (K=128) for h0/h1, then the 17x17 adjacency mix fused
into one small stationary matmul per m-triple group -- no DRAM bounce.

Per core, per 128-row batch tile:
  1. x arrives host-pretransposed/bf16 as xT[j, n, b]; one DMA loads
     [n, j, b]-layout tiles directly (no PE transposes of x).
  2. stage-1 per joint k: one matmul -> PSUM [b, 258], columns
     pre-ordered (t, h, dm) so the whole tile drains in ONE copy into
     h_sb[b, t, h, c3] bf16 (c = h*64 + k*3 + dm, m = 3t+dm)
  3. h_sb pad columns 54:57 hold bias[3t+dm] (written once per buffer),
     so the per-triple PE transposes ([b,128] -> [128, b]) carry bias
     rows into hT for free
  4. one matmul per 4-triple group with stationary bigmix [118, 51]:
     self term + adjacency mix + bias for 51 (j,dm') outputs at once
  5. drain 4 groups per 2-bank PSUM tile into o_sb [128, 2944]; one
     contiguous bf16 store per tile; host un-permutes + upcasts.
"""

import os
import sys

for _p in ("/opt/trn_rl_repo", "/root/.axon_site/_ro/trn_rl_repo"):
    if os.path.isdir(_p) and _p not in sys.path:
        sys.path.insert(0, _p)

import numpy as np

import concourse.bass as bass
import concourse.mybir as mybir
import concourse.tile as tile
from concourse import bacc
from concourse import bass_utils as _bu
from concourse.bass_utils import run_bass_kernel_spmd

B, J, FIN, FOUT = 16384, 17, 128, 128
N_CORES = 8
TB = 128              # batch rows per tile
NT3 = 43              # m-triples per tile (128 = 3*43 - 1; (42,2) is pad)
NG = 11               # groups of <=4 triples: 10 full + 1 of 3 triples
GW = [512] * 10 + [384]          # mix free width per group
GOFF = [512 * g for g in range(11)]
CDIM = 128            # transpose block: c = h*64 + k*3 + dm (+pads)
MIXK = 118            # mix contraction rows (bias rides c=54:57)
OFREE = 6 * 512 - 128  # o_sb free size: 5 paired blocks + 384 tail = 2944
ROWS2 = 128            # o_sb partitions: group pair at rows 0:51 and 64:115
F32 = mybir.dt.float32
BF16 = mybir.dt.bfloat16

_prog_cache: dict[tuple, object] = {}


def _build_program(bs: int, repeat: int = 1):
    """Build the SPMD Bass program for a per-core batch shard of `bs` rows."""
    nt = bs // TB
    assert bs % (2 * TB) == 0, "bs must be a multiple of 256 (paired tiles)"
    np2 = nt // 2

    nc = bacc.Bacc("TRN2", target_bir_lowering=False, debug=False,
                   num_devices=N_CORES)

    xt = nc.declare_dram_parameter("xt", [J, FIN, bs], BF16, isOutput=False)
    wcat = nc.declare_dram_parameter("wcat", [FIN, J, 258], BF16,
                                     isOutput=False)
    bigmix = nc.declare_dram_parameter("bigmix", [MIXK, 51], BF16,
                                       isOutput=False)
    biash = nc.declare_dram_parameter("biash", [TB, NT3, 3], BF16,
                                      isOutput=False)
    ident = nc.declare_dram_parameter("ident", [128, 128], BF16,
                                      isOutput=False)
    outT = nc.declare_dram_parameter("outT", [nt, ROWS2, OFREE], BF16,
                                     isOutput=True)

    with tile.TileContext(nc) as tc:
        with (
            tc.tile_pool(name="const", bufs=1) as cpool,
            tc.tile_pool(name="x", bufs=2) as xpool,
            tc.tile_pool(name="h", bufs=2) as hpool,
            tc.tile_pool(name="hT", bufs=3) as hTpool,
            tc.tile_pool(name="o", bufs=2) as opool,
            tc.tile_pool(name="hk", bufs=2, space=bass.MemorySpace.PSUM) as hkp,
            tc.tile_pool(name="tp", bufs=2, space=bass.MemorySpace.PSUM) as tpp,
            tc.tile_pool(name="mx", bufs=1, space=bass.MemorySpace.PSUM) as mxp,
        ):
            # ---- constants, loaded once ----
            wcat_sb = cpool.tile([FIN, J, 258], BF16, tag="wcat")
            nc.sync.dma_start(wcat_sb[:], wcat[:])
            bigmix_sb = cpool.tile([MIXK, 51], BF16, tag="bigmix")
            nc.sync.dma_start(bigmix_sb[:], bigmix[:])
            id_sb = cpool.tile([128, 128], BF16, tag="ident")
            nc.sync.dma_start(id_sb[:], ident[:])

            # engine-aware PSUM drains (gpsimd can't see PSUM):
            # bf16->bf16 hT drains run 2x on DVE; f32-source drains are
            # 1x everywhere, so spread them to balance busy time.
            def drain(dst, src, kind):
                if kind in ("hT", "hD"):
                    eng = nc.vector.tensor_copy
                else:  # "o" / "hA"
                    eng = nc.scalar.copy
                eng(dst, src)

            def stage1(x_t, half):
                # h_sb[b, t, h, c3], c3 = k*3 + dm; c3 = 51:54 (k=17
                # slot) and 57:64 / h=1 tail stay zero, c3 = 54:57 of
                # h=0 holds bias (both written once below).
                # Joints are processed in pairs: both matmuls of a pair
                # land in one 2-bank PSUM tile and drain in ONE copy.
                h_sb = hpool.tile([TB, NT3, 2, 64], BF16, tag="h")
                for p in range(9):
                    ks = [k for k in (2 * p, 2 * p + 1) if k < J]
                    hk = hkp.tile([TB, 2, 512], F32, tag="hk")
                    for k2, k in enumerate(ks):
                        nc.tensor.matmul(
                            hk[:, k2, 0:258],
                            x_t[:, k, half * TB:(half + 1) * TB],
                            wcat_sb[:, k, :])
                    if len(ks) == 2:
                        dst = h_sb[:, :, :, 6 * p:6 * p + 6].rearrange(
                            "b t h (k2 dm) -> b k2 t h dm", k2=2)
                        drain(dst, hk[:, :, 0:258],
                              "hA" if p in (0, 2, 4, 6, 7) else "hD")
                    else:
                        drain(h_sb[:, :, :, 6 * p:6 * p + 3],
                              hk[:, 0, 0:258], "hD")
                return h_sb

            def mix(h_sb, t_out):
                # mix matmuls lag their group's transposes by one group:
                # the PE chews the next group's transposes while the hT
                # drain completes, instead of stalling on it.
                o_sb = opool.tile([ROWS2, OFREE], BF16, tag="o")
                mps = {}
                pend = None

                def flush(pend):
                    g, hT, w = pend
                    g4 = g // 4
                    if g % 4 == 0:
                        mps[g4] = mxp.tile([ROWS2, 1024], F32, tag="mx",
                                           name="mp")
                    sub, c0 = g % 2, ((g // 2) % 2) * 512
                    nc.tensor.matmul(
                        mps[g4][sub * 64:sub * 64 + 51, c0:c0 + w],
                        bigmix_sb[:], hT[0:MIXK, :w])
                    if g % 4 == 3 or g == NG - 1:
                        bw = c0 + w
                        drain(o_sb[:, g4 * 1024:g4 * 1024 + bw],
                              mps[g4][:, :bw], "o")
                        nc.sync.dma_start(
                            outT[t_out, :, g4 * 1024:g4 * 1024 + bw],
                            o_sb[:, g4 * 1024:g4 * 1024 + bw])

                for g in range(NG):
                    w = GW[g]
                    tp = tpp.tile([CDIM, 512], BF16, tag="tp")
                    for ts in range(w // TB):
                        nc.tensor.transpose(
                            tp[:, ts * TB:(ts + 1) * TB],
                            h_sb[:, 4 * g + ts, :, :], id_sb[:])
                    hT = hTpool.tile([CDIM, 512], BF16, tag="hT")
                    drain(hT[:, :w], tp[:, :w], "hT")
                    if pend is not None:
                        flush(pend)
                    pend = (g, hT, w)
                flush(pend)

            # one-time h-buffer init: zero pads (0*NaN = NaN in the mix
            # matmul otherwise) and plant the bias columns at h=0 c3=54:57
            for _ in range(2):
                hz = hpool.tile([TB, NT3, 2, 64], BF16, tag="h")
                nc.gpsimd.memset(hz[:], 0.0)
                nc.sync.dma_start(hz[:, :, 0, 54:57], biash[:])

            # software pipeline: stage1(t) runs on PE while the copy
            # engines drain t's h; mix(t-1) fills the PE meanwhile.
            prev = None
            for it in range(np2 * repeat):
                p = it % np2
                x_t = xpool.tile([FIN, J, 2 * TB], BF16, tag="x")
                nc.sync.dma_start(
                    x_t[:],
                    xt[:, :, p * 2 * TB:(p + 1) * 2 * TB]
                    .rearrange("j n b -> n j b"))
                for half in (0, 1):
                    h_new = stage1(x_t, half)
                    if prev is not None:
                        mix(*prev)
                    prev = (h_new, 2 * p + half)
            if prev is not None:
                mix(*prev)

    nc.compile()
    return nc


def _host_prep(x, W, bias, adj, bs):
    """Build the per-core input maps (weights-only compute + layout)."""
    import ml_dtypes
    diag = np.diagonal(adj).astype(np.float32)
    off = (adj * (1.0 - np.eye(J, dtype=adj.dtype))).astype(np.float32)

    # stage-1 weights [FIN, J, 258]: col q = t*6 + h*3 + dm holds
    # (h==0 ? diag_k*W0_k : W1_k)[:, 3t+dm], zero at the m=128 pad
    wh = np.zeros((J, FIN, 2, 129), np.float32)
    wh[:, :, 0, :128] = diag[:, None, None] * W[0]
    wh[:, :, 1, :128] = W[1]
    wc = wh.reshape(J, FIN, 2, 43, 3).transpose(0, 1, 3, 2, 4)
    wcat = np.ascontiguousarray(wc.reshape(J, FIN, 258).transpose(1, 0, 2))

    # mix stationary [118, 51]: rows c = h*64 + k*3 + dm (pads zero),
    # rows 54:57 = bias pass-through; cols p = j*3 + dm'
    bm = np.zeros((MIXK, 51), np.float32)
    bm[np.arange(51), np.arange(51)] = 1.0          # h0 self rows
    for dm in range(3):
        for k in range(J):
            bm[64 + 3 * k + dm, dm::3] = off[:, k]  # h1 mix rows
        bm[54 + dm, dm::3] = 1.0                    # bias rows
    # bias plane [TB, 43, 3]: biash[b, t, dm] = bias[3t+dm] (b-bcast)
    mvals = 3 * np.arange(NT3)[:, None] + np.arange(3)[None, :]
    bvals = np.where(mvals < FOUT, bias[np.minimum(mvals, FOUT - 1)], 0.0)
    biash = np.ascontiguousarray(
        np.broadcast_to(bvals[None], (TB, NT3, 3))).astype(np.float32)

    shared = {
        "wcat": wcat.astype(ml_dtypes.bfloat16),
        "bigmix": bm.astype(ml_dtypes.bfloat16),
        "biash": biash.astype(ml_dtypes.bfloat16),
        "ident": np.eye(128, dtype=np.float32).astype(ml_dtypes.bfloat16),
    }
    in_maps = []
    for c in range(N_CORES):
        m = dict(shared)
        xs = x[c * bs:(c + 1) * bs]                  # [bs, J, FIN]
        m["xt"] = np.ascontiguousarray(
            xs.transpose(1, 2, 0)).astype(ml_dtypes.bfloat16)
        in_maps.append(m)
    return in_maps


_decode_idx_cache: dict[int, np.ndarray] = {}


def _decode_idx():
    """Flat gather indices: out[b,j,m] = outT_flat[tile, idx[j,m] + b]."""
    if 0 not in _decode_idx_cache:
        idx = np.zeros((J, FOUT), np.int64)
        for m in range(FOUT):
            t, dm = divmod(m, 3)
            g, ts = divmod(t, 4)
            for j in range(J):
                if g < 10:
                    row = (g % 2) * 64 + j * 3 + dm
                    col = (g // 2) * 512 + ts * TB
                else:
                    row = j * 3 + dm
                    col = 2560 + ts * TB
                idx[j, m] = row * OFREE + col
        _decode_idx_cache[0] = idx
    return _decode_idx_cache[0]


def _decode_out(outT_core, bs):
    """[nt, 102, OFREE] bf16 -> [bs, J, FOUT] f32."""
    nt = bs // TB
    flat = np.ascontiguousarray(outT_core).reshape(nt, ROWS2 * OFREE)
    idx = _decode_idx()                      # [J, FOUT]
    gather = flat[:, idx[None, :, :, None] +
                  np.arange(TB)[None, None, None, :]]  # [nt,1? J,FOUT,TB]
    gather = gather.reshape(nt, J, FOUT, TB)
    return np.ascontiguousarray(
        gather.transpose(0, 3, 1, 2)).reshape(bs, J, FOUT).astype(np.float32)


def _run(x, W, bias, adj, bs, profile=False, tmpdir=None):
    key = (bs,)
    if key not in _prog_cache:
        _prog_cache[key] = _build_program(bs)
    nc = _prog_cache[key]
    in_maps = _host_prep(x, W, bias, adj, bs)
    res = run_bass_kernel_spmd(nc, in_maps, list(range(N_CORES)),
                               trace=profile, tmpdir=tmpdir)
    out = np.concatenate(
        [_decode_out(res.results[c]["outT"], bs) for c in range(N_CORES)],
        axis=0)
    if profile:
        return out, res
    return out


def kernel(x, W, bias, adj):
    x = np.asarray(x, dtype=np.float32)
    W = np.asarray(W, dtype=np.float32)
    bias = np.asarray(bias, dtype=np.float32)
    adj = np.asarray(adj, dtype=np.float32)
    assert x.shape == (B, J, FIN)
    return _run(x, W, bias, adj, B // N_CORES)


# revision 4
# speedup vs baseline: 1.1705x; 1.1705x over previous
"""Trainium2 Bass kernel for DecouplePreAggGraphConv (GNN message passing).

out[b,j,:] = diag(adj)[j] * (x[b,j] @ W0[j])
           + sum_k offdiag(adj)[j,k] * (x[b,k] @ W1[k])
           + bias

Data-parallel over B across 8 NeuronCores. Low-FLOP ("smart") algorithm:
per-joint GEMMs (K=128) for h0/h1, then the 17x17 adjacency mix fused
into one small stationary matmul per m-triple group -- no DRAM bounce.

Per core, per 128-row batch tile:
  1. x arrives host-pretransposed/bf16 as xT[j, n, b]; one DMA loads
     [n, j, b]-layout tiles directly (no PE transposes of x).
  2. stage-1 per joint k: one matmul -> PSUM [b, 258], columns
     pre-ordered (t, h, dm) so the whole tile drains in ONE copy into
     h_sb[b, t, h, c3] bf16 (c = h*64 + k*3 + dm, m = 3t+dm)
  3. h_sb pad columns 54:57 hold bias[3t+dm] (written once per buffer),
     so the per-triple PE transposes ([b,128] -> [128, b]) carry bias
     rows into hT for free
  4. one matmul per 4-triple group with stationary bigmix [118, 51]:
     self term + adjacency mix + bias for 51 (j,dm') outputs at once
  5. drain 4 groups per 2-bank PSUM tile into o_sb [128, 2944]; one
     contiguous bf16 store per tile; host un-permutes + upcasts.
"""

import os
import sys

for _p in ("/opt/trn_rl_repo", "/root/.axon_site/_ro/trn_rl_repo"):
    if os.path.isdir(_p) and _p not in sys.path:
        sys.path.insert(0, _p)

import numpy as np

import concourse.bass as bass
import concourse.mybir as mybir
import concourse.tile as tile
from concourse import bacc
from concourse import bass_utils as _bu
from concourse.bass_utils import run_bass_kernel_spmd

B, J, FIN, FOUT = 16384, 17, 128, 128
N_CORES = 8
TB = 128              # batch rows per tile
NT3 = 43              # m-triples per tile (128 = 3*43 - 1; (42,2) is pad)
NG = 11               # groups of <=4 triples: 10 full + 1 of 3 triples
GW = [512] * 10 + [384]          # mix free width per group
GOFF = [512 * g for g in range(11)]
CDIM = 128            # transpose block: c = h*64 + k*3 + dm (+pads)
MIXK = 118            # mix contraction rows (bias rides c=54:57)
OFREE = 6 * 512 - 128  # o_sb free size: 5 paired blocks + 384 tail = 2944
ROWS2 = 128            # o_sb partitions: group pair at rows 0:51 and 64:115
F32 = mybir.dt.float32
BF16 = mybir.dt.bfloat16

_prog_cache: dict[tuple, object] = {}


def _build_program(bs: int, repeat: int = 1):
    """Build the SPMD Bass program for a per-core batch shard of `bs` rows."""
    nt = bs // TB
    assert bs % (2 * TB) == 0, "bs must be a multiple of 256 (paired tiles)"
    np2 = nt // 2

    nc = bacc.Bacc("TRN2", target_bir_lowering=False, debug=False,
                   num_devices=N_CORES)

    xt = nc.declare_dram_parameter("xt", [J, FIN, bs], BF16, isOutput=False)
    wcat = nc.declare_dram_parameter("wcat", [FIN, J, 258], BF16,
                                     isOutput=False)
    bigmix = nc.declare_dram_parameter("bigmix", [MIXK, 51], BF16,
                                       isOutput=False)
    biash = nc.declare_dram_parameter("biash", [TB, NT3, 3], BF16,
                                      isOutput=False)
    ident = nc.declare_dram_parameter("ident", [128, 128], BF16,
                                      isOutput=False)
    outT = nc.declare_dram_parameter("outT", [nt, ROWS2, OFREE], BF16,
                                     isOutput=True)

    with tile.TileContext(nc) as tc:
        with (
            tc.tile_pool(name="const", bufs=1) as cpool,
            tc.tile_pool(name="x", bufs=3) as xpool,
            tc.tile_pool(name="h", bufs=2) as hpool,
            tc.tile_pool(name="hT", bufs=5) as hTpool,
            tc.tile_pool(name="o", bufs=3) as opool,
            tc.tile_pool(name="hk", bufs=2, space=bass.MemorySpace.PSUM) as hkp,
            tc.tile_pool(name="tp", bufs=2, space=bass.MemorySpace.PSUM) as tpp,
            tc.tile_pool(name="mx", bufs=1, space=bass.MemorySpace.PSUM) as mxp,
        ):
            # ---- constants, loaded once ----
            wcat_sb = cpool.tile([FIN, J, 258], BF16, tag="wcat")
            nc.sync.dma_start(wcat_sb[:], wcat[:])
            bigmix_sb = cpool.tile([MIXK, 51], BF16, tag="bigmix")
            nc.sync.dma_start(bigmix_sb[:], bigmix[:])
            id_sb = cpool.tile([128, 128], BF16, tag="ident")
            nc.sync.dma_start(id_sb[:], ident[:])

            # engine-aware PSUM drains (gpsimd can't see PSUM):
            # bf16->bf16 hT drains run 2x on DVE; f32-source drains are
            # 1x everywhere, so spread them to balance busy time.
            def drain(dst, src, kind):
                if kind in ("hT", "hD"):
                    eng = nc.vector.tensor_copy
                else:  # "o" / "hA"
                    eng = nc.scalar.copy
                eng(dst, src)

            def stage1(x_t, half):
                # h_sb[b, t, h, c3], c3 = k*3 + dm; c3 = 51:54 (k=17
                # slot) and 57:64 / h=1 tail stay zero, c3 = 54:57 of
                # h=0 holds bias (both written once below).
                # Joints are processed in pairs: both matmuls of a pair
                # land in one 2-bank PSUM tile and drain in ONE copy.
                h_sb = hpool.tile([TB, NT3, 2, 64], BF16, tag="h")
                for p in range(9):
                    ks = [k for k in (2 * p, 2 * p + 1) if k < J]
                    hk = hkp.tile([TB, 2, 512], F32, tag="hk")
                    for k2, k in enumerate(ks):
                        nc.tensor.matmul(
                            hk[:, k2, 0:258],
                            x_t[:, k, half * TB:(half + 1) * TB],
                            wcat_sb[:, k, :])
                    if len(ks) == 2:
                        dst = h_sb[:, :, :, 6 * p:6 * p + 6].rearrange(
                            "b t h (k2 dm) -> b k2 t h dm", k2=2)
                        drain(dst, hk[:, :, 0:258],
                              "hA" if p in (0, 2, 4, 6, 7) else "hD")
                    else:
                        drain(h_sb[:, :, :, 6 * p:6 * p + 3],
                              hk[:, 0, 0:258], "hD")
                return h_sb

            def mix(h_sb, t_out):
                # mix matmuls lag their group's transposes by one group:
                # the PE chews the next group's transposes while the hT
                # drain completes, instead of stalling on it.
                o_sb = opool.tile([ROWS2, OFREE], BF16, tag="o")
                mps = {}
                pend = None

                def flush(pend):
                    g, hT, w = pend
                    g4 = g // 4
                    if g % 4 == 0:
                        mps[g4] = mxp.tile([ROWS2, 1024], F32, tag="mx",
                                           name="mp")
                    sub, c0 = g % 2, ((g // 2) % 2) * 512
                    nc.tensor.matmul(
                        mps[g4][sub * 64:sub * 64 + 51, c0:c0 + w],
                        bigmix_sb[:], hT[0:MIXK, :w])
                    if g % 4 == 3 or g == NG - 1:
                        bw = c0 + w
                        drain(o_sb[:, g4 * 1024:g4 * 1024 + bw],
                              mps[g4][:, :bw], "o")
                        nc.sync.dma_start(
                            outT[t_out, :, g4 * 1024:g4 * 1024 + bw],
                            o_sb[:, g4 * 1024:g4 * 1024 + bw])

                for g in range(NG):
                    w = GW[g]
                    tp = tpp.tile([CDIM, 512], BF16, tag="tp")
                    for ts in range(w // TB):
                        nc.tensor.transpose(
                            tp[:, ts * TB:(ts + 1) * TB],
                            h_sb[:, 4 * g + ts, :, :], id_sb[:])
                    hT = hTpool.tile([CDIM, 512], BF16, tag="hT")
                    drain(hT[:, :w], tp[:, :w], "hT")
                    if pend is not None:
                        flush(pend)
                    pend = (g, hT, w)
                flush(pend)

            # one-time h-buffer init: zero pads (0*NaN = NaN in the mix
            # matmul otherwise) and plant the bias columns at h=0 c3=54:57
            for _ in range(2):
                hz = hpool.tile([TB, NT3, 2, 64], BF16, tag="h")
                nc.gpsimd.memset(hz[:], 0.0)
                nc.sync.dma_start(hz[:, :, 0, 54:57], biash[:])

            # software pipeline: stage1(t) runs on PE while the copy
            # engines drain t's h; mix(t-1) fills the PE meanwhile.
            prev = None
            for it in range(np2 * repeat):
                p = it % np2
                x_t = xpool.tile([FIN, J, 2 * TB], BF16, tag="x")
                nc.sync.dma_start(
                    x_t[:],
                    xt[:, :, p * 2 * TB:(p + 1) * 2 * TB]
                    .rearrange("j n b -> n j b"))
                for half in (0, 1):
                    h_new = stage1(x_t, half)
                    if prev is not None:
                        mix(*prev)
                    prev = (h_new, 2 * p + half)
            if prev is not None:
                mix(*prev)

    nc.compile()
    return nc


def _host_prep(x, W, bias, adj, bs):
    """Build the per-core input maps (weights-only compute + layout)."""
    import ml_dtypes
    diag = np.diagonal(adj).astype(np.float32)
    off = (adj * (1.0 - np.eye(J, dtype=adj.dtype))).astype(np.float32)

    # stage-1 weights [FIN, J, 258]: col q = t*6 + h*3 + dm holds
    # (h==0 ? diag_k*W0_k : W1_k)[:, 3t+dm], zero at the m=128 pad
    wh = np.zeros((J, FIN, 2, 129), np.float32)
    wh[:, :, 0, :128] = diag[:, None, None] * W[0]
    wh[:, :, 1, :128] = W[1]
    wc = wh.reshape(J, FIN, 2, 43, 3).transpose(0, 1, 3, 2, 4)
    wcat = np.ascontiguousarray(wc.reshape(J, FIN, 258).transpose(1, 0, 2))

    # mix stationary [118, 51]: rows c = h*64 + k*3 + dm (pads zero),
    # rows 54:57 = bias pass-through; cols p = j*3 + dm'
    bm = np.zeros((MIXK, 51), np.float32)
    bm[np.arange(51), np.arange(51)] = 1.0          # h0 self rows
    for dm in range(3):
        for k in range(J):
            bm[64 + 3 * k + dm, dm::3] = off[:, k]  # h1 mix rows
        bm[54 + dm, dm::3] = 1.0                    # bias rows
    # bias plane [TB, 43, 3]: biash[b, t, dm] = bias[3t+dm] (b-bcast)
    mvals = 3 * np.arange(NT3)[:, None] + np.arange(3)[None, :]
    bvals = np.where(mvals < FOUT, bias[np.minimum(mvals, FOUT - 1)], 0.0)
    biash = np.ascontiguousarray(
        np.broadcast_to(bvals[None], (TB, NT3, 3))).astype(np.float32)

    shared = {
        "wcat": wcat.astype(ml_dtypes.bfloat16),
        "bigmix": bm.astype(ml_dtypes.bfloat16),
        "biash": biash.astype(ml_dtypes.bfloat16),
        "ident": np.eye(128, dtype=np.float32).astype(ml_dtypes.bfloat16),
    }
    in_maps = []
    for c in range(N_CORES):
        m = dict(shared)
        xs = x[c * bs:(c + 1) * bs]                  # [bs, J, FIN]
        m["xt"] = np.ascontiguousarray(
            xs.transpose(1, 2, 0)).astype(ml_dtypes.bfloat16)
        in_maps.append(m)
    return in_maps


_decode_idx_cache: dict[int, np.ndarray] = {}


def _decode_idx():
    """Flat gather indices: out[b,j,m] = outT_flat[tile, idx[j,m] + b]."""
    if 0 not in _decode_idx_cache:
        idx = np.zeros((J, FOUT), np.int64)
        for m in range(FOUT):
            t, dm = divmod(m, 3)
            g, ts = divmod(t, 4)
            for j in range(J):
                if g < 10:
                    row = (g % 2) * 64 + j * 3 + dm
                    col = (g // 2) * 512 + ts * TB
                else:
                    row = j * 3 + dm
                    col = 2560 + ts * TB
                idx[j, m] = row * OFREE + col
        _decode_idx_cache[0] = idx
    return _decode_idx_cache[0]


def _decode_out(outT_core, bs):
    """[nt, 102, OFREE] bf16 -> [bs, J, FOUT] f32."""
    nt = bs // TB
    flat = np.ascontiguousarray(outT_core).reshape(nt, ROWS2 * OFREE)
    idx = _decode_idx()                      # [J, FOUT]
    gather = flat[:, idx[None, :, :, None] +
                  np.arange(TB)[None, None, None, :]]  # [nt,1? J,FOUT,TB]
    gather = gather.reshape(nt, J, FOUT, TB)
    return np.ascontiguousarray(
        gather.transpose(0, 3, 1, 2)).reshape(bs, J, FOUT).astype(np.float32)


def _run(x, W, bias, adj, bs, profile=False, tmpdir=None):
    key = (bs,)
    if key not in _prog_cache:
        _prog_cache[key] = _build_program(bs)
    nc = _prog_cache[key]
    in_maps = _host_prep(x, W, bias, adj, bs)
    res = run_bass_kernel_spmd(nc, in_maps, list(range(N_CORES)),
                               trace=profile, tmpdir=tmpdir)
    out = np.concatenate(
        [_decode_out(res.results[c]["outT"], bs) for c in range(N_CORES)],
        axis=0)
    if profile:
        return out, res
    return out


def kernel(x, W, bias, adj):
    x = np.asarray(x, dtype=np.float32)
    W = np.asarray(W, dtype=np.float32)
    bias = np.asarray(bias, dtype=np.float32)
    adj = np.asarray(adj, dtype=np.float32)
    assert x.shape == (B, J, FIN)
    return _run(x, W, bias, adj, B // N_CORES)


# revision 5
# speedup vs baseline: 1.3037x; 1.1138x over previous
"""Trainium2 Bass kernel for DecouplePreAggGraphConv (GNN message passing).

out[b,j,:] = diag(adj)[j] * (x[b,j] @ W0[j])
           + sum_k offdiag(adj)[j,k] * (x[b,k] @ W1[k])
           + bias

Data-parallel over B across 8 NeuronCores. Low-FLOP ("smart") algorithm:
per-joint GEMMs (K=128) for h0/h1, then the 17x17 adjacency mix fused
into one small stationary matmul per m-triple group -- no DRAM bounce.

Per core, per 128-row batch tile:
  1. x arrives host-pretransposed/bf16 as xT[j, n, b]; one DMA loads
     [n, j, b]-layout tiles directly (no PE transposes of x).
  2. stage-1 per joint k: one matmul -> PSUM [b, 258], columns
     pre-ordered (t, h, dm) so the whole tile drains in ONE copy into
     h_sb[b, t, h, c3] bf16 (c = h*64 + k*3 + dm, m = 3t+dm)
  3. h_sb pad columns 54:57 hold bias[3t+dm] (written once per buffer),
     so the per-triple PE transposes ([b,128] -> [128, b]) carry bias
     rows into hT for free
  4. one matmul per 4-triple group with stationary bigmix [118, 51]:
     self term + adjacency mix + bias for 51 (j,dm') outputs at once
  5. drain 4 groups per 2-bank PSUM tile into o_sb [128, 2944]; one
     contiguous bf16 store per tile; host un-permutes + upcasts.
"""

import os
import sys

for _p in ("/opt/trn_rl_repo", "/root/.axon_site/_ro/trn_rl_repo"):
    if os.path.isdir(_p) and _p not in sys.path:
        sys.path.insert(0, _p)

import numpy as np

import concourse.bass as bass
import concourse.mybir as mybir
import concourse.tile as tile
from concourse import bacc
from concourse import bass_utils as _bu
from concourse.bass_utils import run_bass_kernel_spmd

B, J, FIN, FOUT = 16384, 17, 128, 128
N_CORES = 8
TB = 128              # batch rows per tile
NT3 = 43              # m-triples per tile (128 = 3*43 - 1; (42,2) is pad)
NG = 11               # groups of <=4 triples: 10 full + 1 of 3 triples
GW = [512] * 10 + [384]          # mix free width per group
CDIM = 128            # transpose block: c = h*64 + k*3 + dm (+pads)
MIXK = 118            # mix contraction rows (bias rides c=54:57)
OFREE = 6 * 512 - 128  # o_sb free size: 5 paired blocks + 384 tail = 2944
ROWS2 = 128            # o_sb partitions: group pair at rows 0:51 and 64:115
F32 = mybir.dt.float32
BF16 = mybir.dt.bfloat16

_prog_cache: dict[tuple, object] = {}


def _build_program(bs: int, repeat: int = 1):
    """Build the SPMD Bass program for a per-core batch shard of `bs` rows."""
    nt = bs // TB
    assert bs % (2 * TB) == 0, "bs must be a multiple of 256 (paired tiles)"
    np2 = nt // 2

    nc = bacc.Bacc("TRN2", target_bir_lowering=False, debug=False,
                   num_devices=N_CORES)

    xt = nc.declare_dram_parameter("xt", [J, FIN, bs], BF16, isOutput=False)
    wcat = nc.declare_dram_parameter("wcat", [FIN, J, 258], BF16,
                                     isOutput=False)
    bigmix = nc.declare_dram_parameter("bigmix", [MIXK, 51], BF16,
                                       isOutput=False)
    biash = nc.declare_dram_parameter("biash", [TB, NT3, 3], BF16,
                                      isOutput=False)
    ident = nc.declare_dram_parameter("ident", [128, 128], BF16,
                                      isOutput=False)
    outT = nc.declare_dram_parameter("outT", [nt, ROWS2, OFREE], BF16,
                                     isOutput=True)

    with tile.TileContext(nc) as tc:
        with (
            tc.tile_pool(name="const", bufs=1) as cpool,
            tc.tile_pool(name="x", bufs=3) as xpool,
            tc.tile_pool(name="h", bufs=2) as hpool,
            tc.tile_pool(name="hT", bufs=5) as hTpool,
            tc.tile_pool(name="o", bufs=3) as opool,
            tc.tile_pool(name="hk", bufs=2, space=bass.MemorySpace.PSUM) as hkp,
            tc.tile_pool(name="tp", bufs=2, space=bass.MemorySpace.PSUM) as tpp,
            tc.tile_pool(name="mx", bufs=1, space=bass.MemorySpace.PSUM) as mxp,
        ):
            # ---- constants, loaded once ----
            wcat_sb = cpool.tile([FIN, J, 258], BF16, tag="wcat")
            nc.sync.dma_start(wcat_sb[:], wcat[:])
            bigmix_sb = cpool.tile([MIXK, 51], BF16, tag="bigmix")
            nc.sync.dma_start(bigmix_sb[:], bigmix[:])
            id_sb = cpool.tile([128, 128], BF16, tag="ident")
            nc.sync.dma_start(id_sb[:], ident[:])

            # engine-aware PSUM drains (gpsimd can't see PSUM):
            # bf16->bf16 hT drains run 2x on DVE; f32-source drains are
            # 1x everywhere, so spread them to balance busy time.
            def drain(dst, src, kind):
                if kind in ("hT", "hD"):
                    eng = nc.vector.tensor_copy
                else:  # "o" / "hA"
                    eng = nc.scalar.copy
                eng(dst, src)

            def stage1(x_t, half):
                # h_sb[b, t, h, c3], c3 = k*3 + dm; c3 = 51:54 (k=17
                # slot) and 57:64 / h=1 tail stay zero, c3 = 54:57 of
                # h=0 holds bias (both written once below).
                # Joints are processed in pairs: both matmuls of a pair
                # land in one 2-bank PSUM tile and drain in ONE copy.
                h_sb = hpool.tile([TB, NT3, 2, 64], BF16, tag="h")
                for p in range(9):
                    ks = [k for k in (2 * p, 2 * p + 1) if k < J]
                    hk = hkp.tile([TB, 2, 512], F32, tag="hk")
                    for k2, k in enumerate(ks):
                        nc.tensor.matmul(
                            hk[:, k2, 0:258],
                            x_t[:, k, half * TB:(half + 1) * TB],
                            wcat_sb[:, k, :])
                    if len(ks) == 2:
                        dst = h_sb[:, :, :, 6 * p:6 * p + 6].rearrange(
                            "b t h (k2 dm) -> b k2 t h dm", k2=2)
                        drain(dst, hk[:, :, 0:258],
                              "hA" if p in (0, 2, 4, 6, 7) else "hD")
                    else:
                        drain(h_sb[:, :, :, 6 * p:6 * p + 3],
                              hk[:, 0, 0:258], "hD")
                return h_sb

            def mix(h_sb, t_out):
                # mix matmuls lag their group's transposes by one group:
                # the PE chews the next group's transposes while the hT
                # drain completes, instead of stalling on it.
                o_sb = opool.tile([ROWS2, OFREE], BF16, tag="o")
                mps = {}
                pend = None

                def flush(pend):
                    g, hT, w = pend
                    g4 = g // 4
                    if g % 4 == 0:
                        mps[g4] = mxp.tile([ROWS2, 1024], F32, tag="mx",
                                           name="mp")
                    sub, c0 = g % 2, ((g // 2) % 2) * 512
                    nc.tensor.matmul(
                        mps[g4][sub * 64:sub * 64 + 51, c0:c0 + w],
                        bigmix_sb[:], hT[0:MIXK, :w])
                    if g % 4 == 3 or g == NG - 1:
                        bw = c0 + w
                        drain(o_sb[:, g4 * 1024:g4 * 1024 + bw],
                              mps[g4][:, :bw], "o")
                        nc.sync.dma_start(
                            outT[t_out, :, g4 * 1024:g4 * 1024 + bw],
                            o_sb[:, g4 * 1024:g4 * 1024 + bw])

                for g in range(NG):
                    w = GW[g]
                    tp = tpp.tile([CDIM, 512], BF16, tag="tp")
                    for ts in range(w // TB):
                        nc.tensor.transpose(
                            tp[:, ts * TB:(ts + 1) * TB],
                            h_sb[:, 4 * g + ts, :, :], id_sb[:])
                    hT = hTpool.tile([CDIM, 512], BF16, tag="hT")
                    drain(hT[:, :w], tp[:, :w], "hT")
                    if pend is not None:
                        flush(pend)
                    pend = (g, hT, w)
                flush(pend)

            # one-time h-buffer init: zero pads (0*NaN = NaN in the mix
            # matmul otherwise) and plant the bias columns at h=0 c3=54:57
            for _ in range(2):
                hz = hpool.tile([TB, NT3, 2, 64], BF16, tag="h")
                nc.gpsimd.memset(hz[:], 0.0)
                nc.sync.dma_start(hz[:, :, 0, 54:57], biash[:])

            # software pipeline: stage1(t) runs on PE while the copy
            # engines drain t's h; mix(t-1) fills the PE meanwhile.
            prev = None
            for it in range(np2 * repeat):
                p = it % np2
                x_t = xpool.tile([FIN, J, 2 * TB], BF16, tag="x")
                nc.sync.dma_start(
                    x_t[:],
                    xt[:, :, p * 2 * TB:(p + 1) * 2 * TB]
                    .rearrange("j n b -> n j b"))
                for half in (0, 1):
                    h_new = stage1(x_t, half)
                    if prev is not None:
                        mix(*prev)
                    prev = (h_new, 2 * p + half)
            if prev is not None:
                mix(*prev)

    nc.compile()
    return nc


def _host_prep(x, W, bias, adj, bs):
    """Build the per-core input maps (weights-only compute + layout)."""
    import ml_dtypes
    diag = np.diagonal(adj).astype(np.float32)
    off = (adj * (1.0 - np.eye(J, dtype=adj.dtype))).astype(np.float32)

    # stage-1 weights [FIN, J, 258]: col q = t*6 + h*3 + dm holds
    # (h==0 ? diag_k*W0_k : W1_k)[:, 3t+dm], zero at the m=128 pad
    wh = np.zeros((J, FIN, 2, 129), np.float32)
    wh[:, :, 0, :128] = diag[:, None, None] * W[0]
    wh[:, :, 1, :128] = W[1]
    wc = wh.reshape(J, FIN, 2, 43, 3).transpose(0, 1, 3, 2, 4)
    wcat = np.ascontiguousarray(wc.reshape(J, FIN, 258).transpose(1, 0, 2))

    # mix stationary [118, 51]: rows c = h*64 + k*3 + dm (pads zero),
    # rows 54:57 = bias pass-through; cols p = j*3 + dm'
    bm = np.zeros((MIXK, 51), np.float32)
    bm[np.arange(51), np.arange(51)] = 1.0          # h0 self rows
    for dm in range(3):
        for k in range(J):
            bm[64 + 3 * k + dm, dm::3] = off[:, k]  # h1 mix rows
        bm[54 + dm, dm::3] = 1.0                    # bias rows
    # bias plane [TB, 43, 3]: biash[b, t, dm] = bias[3t+dm] (b-bcast)
    mvals = 3 * np.arange(NT3)[:, None] + np.arange(3)[None, :]
    bvals = np.where(mvals < FOUT, bias[np.minimum(mvals, FOUT - 1)], 0.0)
    biash = np.ascontiguousarray(
        np.broadcast_to(bvals[None], (TB, NT3, 3))).astype(np.float32)

    shared = {
        "wcat": wcat.astype(ml_dtypes.bfloat16),
        "bigmix": bm.astype(ml_dtypes.bfloat16),
        "biash": biash.astype(ml_dtypes.bfloat16),
        "ident": np.eye(128, dtype=np.float32).astype(ml_dtypes.bfloat16),
    }
    in_maps = []
    for c in range(N_CORES):
        m = dict(shared)
        xs = x[c * bs:(c + 1) * bs]                  # [bs, J, FIN]
        m["xt"] = np.ascontiguousarray(
            xs.transpose(1, 2, 0)).astype(ml_dtypes.bfloat16)
        in_maps.append(m)
    return in_maps


_decode_idx_cache: dict[int, np.ndarray] = {}


def _decode_idx():
    """Flat gather indices: out[b,j,m] = outT_flat[tile, idx[j,m] + b]."""
    if 0 not in _decode_idx_cache:
        idx = np.zeros((J, FOUT), np.int64)
        for m in range(FOUT):
            t, dm = divmod(m, 3)
            g, ts = divmod(t, 4)
            for j in range(J):
                if g < 10:
                    row = (g % 2) * 64 + j * 3 + dm
                    col = (g // 2) * 512 + ts * TB
                else:
                    row = j * 3 + dm
                    col = 2560 + ts * TB
                idx[j, m] = row * OFREE + col
        _decode_idx_cache[0] = idx
    return _decode_idx_cache[0]


def _decode_out(outT_core, bs):
    """[nt, 102, OFREE] bf16 -> [bs, J, FOUT] f32."""
    nt = bs // TB
    flat = np.ascontiguousarray(outT_core).reshape(nt, ROWS2 * OFREE)
    idx = _decode_idx()                      # [J, FOUT]
    gather = flat[:, idx[None, :, :, None] +
                  np.arange(TB)[None, None, None, :]]  # [nt,1? J,FOUT,TB]
    gather = gather.reshape(nt, J, FOUT, TB)
    return np.ascontiguousarray(
        gather.transpose(0, 3, 1, 2)).reshape(bs, J, FOUT).astype(np.float32)


def _run(x, W, bias, adj, bs, profile=False, tmpdir=None):
    key = (bs,)
    if key not in _prog_cache:
        _prog_cache[key] = _build_program(bs)
    nc = _prog_cache[key]
    in_maps = _host_prep(x, W, bias, adj, bs)
    res = run_bass_kernel_spmd(nc, in_maps, list(range(N_CORES)),
                               trace=profile, tmpdir=tmpdir)
    out = np.concatenate(
        [_decode_out(res.results[c]["outT"], bs) for c in range(N_CORES)],
        axis=0)
    if profile:
        return out, res
    return out


def kernel(x, W, bias, adj):
    x = np.asarray(x, dtype=np.float32)
    W = np.asarray(W, dtype=np.float32)
    bias = np.asarray(bias, dtype=np.float32)
    adj = np.asarray(adj, dtype=np.float32)
    assert x.shape == (B, J, FIN)
    return _run(x, W, bias, adj, B // N_CORES)


# revision 6
# speedup vs baseline: 1.3935x; 1.0689x over previous
"""Trainium2 Bass kernel for DecouplePreAggGraphConv (GNN message passing).

out[b,j,:] = diag(adj)[j] * (x[b,j] @ W0[j])
           + sum_k offdiag(adj)[j,k] * (x[b,k] @ W1[k])
           + bias

Data-parallel over B across 8 NeuronCores. Low-FLOP ("smart") algorithm:
per-joint GEMMs (K=128) for h0/h1, then the 17x17 adjacency mix fused
into one small stationary matmul per m-triple group -- no DRAM bounce.

Per core, per 128-row batch tile:
  1. x arrives host-pretransposed/bf16 as xT[j, n, b]; one DMA loads
     [n, j, b]-layout tiles directly (no PE transposes of x).
  2. stage-1 per joint k: one matmul -> PSUM [b, 258], columns
     pre-ordered (t, h, dm) so the whole tile drains in ONE copy into
     h_sb[b, t, h, c3] bf16 (c = h*64 + k*3 + dm, m = 3t+dm)
  3. h_sb pad columns 54:57 hold bias[3t+dm] (written once per buffer),
     so the per-triple PE transposes ([b,128] -> [128, b]) carry bias
     rows into hT for free
  4. one matmul per 4-triple group with stationary bigmix [118, 51]:
     self term + adjacency mix + bias for 51 (j,dm') outputs at once
     (lagged one group behind its transposes so the PE never stalls)
  5. drain 4 groups per 2-bank PSUM tile into o_sb [128, 2944] slices;
     three bf16 stores per tile; host un-permutes + upcasts.
"""

import os
import sys

for _p in ("/opt/trn_rl_repo", "/root/.axon_site/_ro/trn_rl_repo"):
    if os.path.isdir(_p) and _p not in sys.path:
        sys.path.insert(0, _p)

import numpy as np

import concourse.bass as bass
import concourse.mybir as mybir
import concourse.tile as tile
from concourse import bacc
from concourse import bass_utils as _bu
from concourse.bass_utils import run_bass_kernel_spmd

B, J, FIN, FOUT = 16384, 17, 128, 128
N_CORES = 8
TB = 128              # batch rows per tile
NT3 = 43              # m-triples per tile (128 = 3*43 - 1; (42,2) is pad)
NG = 11               # groups of <=4 triples: 10 full + 1 of 3 triples
GW = [512] * 10 + [384]          # mix free width per group
CDIM = 128            # transpose block: c = h*64 + k*3 + dm (+pads)
MIXK = 118            # mix contraction rows (bias rides c=54:57)
OFREE = 6 * 512 - 128  # o_sb free size: 5 paired blocks + 384 tail = 2944
ROWS2 = 128            # o_sb partitions: group pair at rows 0:51 and 64:115
F32 = mybir.dt.float32
BF16 = mybir.dt.bfloat16

_prog_cache: dict[tuple, object] = {}


def _build_program(bs: int, repeat: int = 1):
    """Build the SPMD Bass program for a per-core batch shard of `bs` rows."""
    nt = bs // TB
    assert bs % (2 * TB) == 0, "bs must be a multiple of 256 (paired tiles)"
    np2 = nt // 2

    nc = bacc.Bacc("TRN2", target_bir_lowering=False, debug=False,
                   num_devices=N_CORES)

    xt = nc.declare_dram_parameter("xt", [J, FIN, bs], BF16, isOutput=False)
    wcat = nc.declare_dram_parameter("wcat", [FIN, J, 258], BF16,
                                     isOutput=False)
    bigmix = nc.declare_dram_parameter("bigmix", [MIXK, 51], BF16,
                                       isOutput=False)
    biash = nc.declare_dram_parameter("biash", [TB, NT3, 3], BF16,
                                      isOutput=False)
    ident = nc.declare_dram_parameter("ident", [128, 128], BF16,
                                      isOutput=False)
    outT = nc.declare_dram_parameter("outT", [nt, ROWS2, OFREE], BF16,
                                     isOutput=True)

    with tile.TileContext(nc) as tc:
        with (
            tc.tile_pool(name="const", bufs=1) as cpool,
            tc.tile_pool(name="x", bufs=3) as xpool,
            tc.tile_pool(name="h", bufs=2) as hpool,
            tc.tile_pool(name="hT", bufs=5) as hTpool,
            tc.tile_pool(name="o", bufs=3) as opool,
            tc.tile_pool(name="hk", bufs=2, space=bass.MemorySpace.PSUM) as hkp,
            tc.tile_pool(name="tp", bufs=2, space=bass.MemorySpace.PSUM) as tpp,
            tc.tile_pool(name="mx", bufs=1, space=bass.MemorySpace.PSUM) as mxp,
        ):
            # ---- constants, loaded once ----
            wcat_sb = cpool.tile([FIN, J, 258], BF16, tag="wcat")
            nc.sync.dma_start(wcat_sb[:], wcat[:])
            bigmix_sb = cpool.tile([MIXK, 51], BF16, tag="bigmix")
            nc.sync.dma_start(bigmix_sb[:], bigmix[:])
            id_sb = cpool.tile([128, 128], BF16, tag="ident")
            nc.sync.dma_start(id_sb[:], ident[:])

            # engine-aware PSUM drains (gpsimd can't see PSUM):
            # bf16->bf16 hT drains run 2x on DVE; f32-source drains are
            # 1x everywhere, so spread them to balance busy time.
            def drain(dst, src, kind):
                if kind in ("hT", "hD"):
                    eng = nc.vector.tensor_copy
                else:  # "o" / "hA"
                    eng = nc.scalar.copy
                eng(dst, src)

            def stage1(x_t, half):
                # h_sb[b, t, h, c3], c3 = k*3 + dm; c3 = 51:54 (k=17
                # slot) and 57:64 / h=1 tail stay zero, c3 = 54:57 of
                # h=0 holds bias (both written once below).
                # Joints are processed in pairs: both matmuls of a pair
                # land in one 2-bank PSUM tile and drain in ONE copy.
                h_sb = hpool.tile([TB, NT3, 2, 64], BF16, tag="h")
                for p in range(9):
                    ks = [k for k in (2 * p, 2 * p + 1) if k < J]
                    hk = hkp.tile([TB, 2, 512], F32, tag="hk")
                    for k2, k in enumerate(ks):
                        nc.tensor.matmul(
                            hk[:, k2, 0:258],
                            x_t[:, k, half * TB:(half + 1) * TB],
                            wcat_sb[:, k, :])
                    if len(ks) == 2:
                        dst = h_sb[:, :, :, 6 * p:6 * p + 6].rearrange(
                            "b t h (k2 dm) -> b k2 t h dm", k2=2)
                        drain(dst, hk[:, :, 0:258],
                              "hA" if p in (0, 2, 4, 6, 7) else "hD")
                    else:
                        drain(h_sb[:, :, :, 6 * p:6 * p + 3],
                              hk[:, 0, 0:258], "hD")
                return h_sb

            def mix(h_sb, t_out):
                # mix matmuls lag their group's transposes by one group:
                # the PE chews the next group's transposes while the hT
                # drain completes, instead of stalling on it.
                o_sb = opool.tile([ROWS2, OFREE], BF16, tag="o")
                mps = {}
                pend = None

                def flush(pend):
                    g, hT, w = pend
                    g4 = g // 4
                    if g % 4 == 0:
                        mps[g4] = mxp.tile([ROWS2, 1024], F32, tag="mx",
                                           name="mp")
                    sub, c0 = g % 2, ((g // 2) % 2) * 512
                    nc.tensor.matmul(
                        mps[g4][sub * 64:sub * 64 + 51, c0:c0 + w],
                        bigmix_sb[:], hT[0:MIXK, :w])
                    if g % 4 == 3 or g == NG - 1:
                        bw = c0 + w
                        drain(o_sb[:, g4 * 1024:g4 * 1024 + bw],
                              mps[g4][:, :bw], "o")
                        nc.sync.dma_start(
                            outT[t_out, :, g4 * 1024:g4 * 1024 + bw],
                            o_sb[:, g4 * 1024:g4 * 1024 + bw])

                for g in range(NG):
                    w = GW[g]
                    tp = tpp.tile([CDIM, 512], BF16, tag="tp")
                    for ts in range(w // TB):
                        nc.tensor.transpose(
                            tp[:, ts * TB:(ts + 1) * TB],
                            h_sb[:, 4 * g + ts, :, :], id_sb[:])
                    hT = hTpool.tile([CDIM, 512], BF16, tag="hT")
                    drain(hT[:, :w], tp[:, :w], "hT")
                    if pend is not None:
                        flush(pend)
                    pend = (g, hT, w)
                flush(pend)

            # one-time h-buffer init: zero pads (0*NaN = NaN in the mix
            # matmul otherwise) and plant the bias columns at h=0 c3=54:57
            for _ in range(2):
                hz = hpool.tile([TB, NT3, 2, 64], BF16, tag="h")
                nc.gpsimd.memset(hz[:], 0.0)
                nc.sync.dma_start(hz[:, :, 0, 54:57], biash[:])

            # software pipeline: stage1(t) runs on PE while the copy
            # engines drain t's h; mix(t-1) fills the PE meanwhile.
            prev = None
            for it in range(np2 * repeat):
                p = it % np2
                x_t = xpool.tile([FIN, J, 2 * TB], BF16, tag="x")
                nc.sync.dma_start(
                    x_t[:],
                    xt[:, :, p * 2 * TB:(p + 1) * 2 * TB]
                    .rearrange("j n b -> n j b"))
                for half in (0, 1):
                    h_new = stage1(x_t, half)
                    if prev is not None:
                        mix(*prev)
                    prev = (h_new, 2 * p + half)
            if prev is not None:
                mix(*prev)

    nc.compile()
    return nc


def _host_prep(x, W, bias, adj, bs):
    """Build the per-core input maps (weights-only compute + layout)."""
    import ml_dtypes
    diag = np.diagonal(adj).astype(np.float32)
    off = (adj * (1.0 - np.eye(J, dtype=adj.dtype))).astype(np.float32)

    # stage-1 weights [FIN, J, 258]: col q = t*6 + h*3 + dm holds
    # (h==0 ? diag_k*W0_k : W1_k)[:, 3t+dm], zero at the m=128 pad
    wh = np.zeros((J, FIN, 2, 129), np.float32)
    wh[:, :, 0, :128] = diag[:, None, None] * W[0]
    wh[:, :, 1, :128] = W[1]
    wc = wh.reshape(J, FIN, 2, 43, 3).transpose(0, 1, 3, 2, 4)
    wcat = np.ascontiguousarray(wc.reshape(J, FIN, 258).transpose(1, 0, 2))

    # mix stationary [118, 51]: rows c = h*64 + k*3 + dm (pads zero),
    # rows 54:57 = bias pass-through; cols p = j*3 + dm'
    bm = np.zeros((MIXK, 51), np.float32)
    bm[np.arange(51), np.arange(51)] = 1.0          # h0 self rows
    for dm in range(3):
        for k in range(J):
            bm[64 + 3 * k + dm, dm::3] = off[:, k]  # h1 mix rows
        bm[54 + dm, dm::3] = 1.0                    # bias rows
    # bias plane [TB, 43, 3]: biash[b, t, dm] = bias[3t+dm] (b-bcast)
    mvals = 3 * np.arange(NT3)[:, None] + np.arange(3)[None, :]
    bvals = np.where(mvals < FOUT, bias[np.minimum(mvals, FOUT - 1)], 0.0)
    biash = np.ascontiguousarray(
        np.broadcast_to(bvals[None], (TB, NT3, 3))).astype(np.float32)

    shared = {
        "wcat": wcat.astype(ml_dtypes.bfloat16),
        "bigmix": bm.astype(ml_dtypes.bfloat16),
        "biash": biash.astype(ml_dtypes.bfloat16),
        "ident": np.eye(128, dtype=np.float32).astype(ml_dtypes.bfloat16),
    }
    in_maps = []
    for c in range(N_CORES):
        m = dict(shared)
        xs = x[c * bs:(c + 1) * bs]                  # [bs, J, FIN]
        m["xt"] = np.ascontiguousarray(
            xs.transpose(1, 2, 0)).astype(ml_dtypes.bfloat16)
        in_maps.append(m)
    return in_maps


_decode_idx_cache: dict[int, np.ndarray] = {}


def _decode_idx():
    """Flat gather indices: out[b,j,m] = outT_flat[tile, idx[j,m] + b]."""
    if 0 not in _decode_idx_cache:
        idx = np.zeros((J, FOUT), np.int64)
        for m in range(FOUT):
            t, dm = divmod(m, 3)
            g, ts = divmod(t, 4)
            for j in range(J):
                if g < 10:
                    row = (g % 2) * 64 + j * 3 + dm
                    col = (g // 2) * 512 + ts * TB
                else:
                    row = j * 3 + dm
                    col = 2560 + ts * TB
                idx[j, m] = row * OFREE + col
        _decode_idx_cache[0] = idx
    return _decode_idx_cache[0]


def _decode_out(outT_core, bs):
    """[nt, 102, OFREE] bf16 -> [bs, J, FOUT] f32."""
    nt = bs // TB
    flat = np.ascontiguousarray(outT_core).reshape(nt, ROWS2 * OFREE)
    idx = _decode_idx()                      # [J, FOUT]
    gather = flat[:, idx[None, :, :, None] +
                  np.arange(TB)[None, None, None, :]]  # [nt,1? J,FOUT,TB]
    gather = gather.reshape(nt, J, FOUT, TB)
    return np.ascontiguousarray(
        gather.transpose(0, 3, 1, 2)).reshape(bs, J, FOUT).astype(np.float32)


def _run(x, W, bias, adj, bs, profile=False, tmpdir=None):
    key = (bs,)
    if key not in _prog_cache:
        _prog_cache[key] = _build_program(bs)
    nc = _prog_cache[key]
    in_maps = _host_prep(x, W, bias, adj, bs)
    res = run_bass_kernel_spmd(nc, in_maps, list(range(N_CORES)),
                               trace=profile, tmpdir=tmpdir)
    out = np.concatenate(
        [_decode_out(res.results[c]["outT"], bs) for c in range(N_CORES)],
        axis=0)
    if profile:
        return out, res
    return out


def kernel(x, W, bias, adj):
    x = np.asarray(x, dtype=np.float32)
    W = np.asarray(W, dtype=np.float32)
    bias = np.asarray(bias, dtype=np.float32)
    adj = np.asarray(adj, dtype=np.float32)
    assert x.shape == (B, J, FIN)
    return _run(x, W, bias, adj, B // N_CORES)


# revision 7
# speedup vs baseline: 1.5423x; 1.1068x over previous
"""Trainium2 Bass kernel for DecouplePreAggGraphConv (GNN message passing).

out[b,j,:] = diag(adj)[j] * (x[b,j] @ W0[j])
           + sum_k offdiag(adj)[j,k] * (x[b,k] @ W1[k])
           + bias

Data-parallel over B across 8 NeuronCores. Low-FLOP ("smart") algorithm:
per-joint GEMMs (K=128) for h0/h1, then the 17x17 adjacency mix fused
into one small stationary matmul per m-triple group -- no DRAM bounce.

Per core, per 128-row batch tile:
  1. x arrives host-pretransposed/bf16 as xT[j, n, b]; one DMA loads
     [n, j, b]-layout tiles directly (no PE transposes of x).
  2. stage-1 per joint k: one matmul -> PSUM [b, 258], columns
     pre-ordered (t, h, dm) so the whole tile drains in ONE copy into
     h_sb[b, t, h, c3] bf16 (c = h*64 + k*3 + dm, m = 3t+dm)
  3. h_sb pad columns 54:57 hold bias[3t+dm] (written once per buffer),
     so the per-triple PE transposes ([b,128] -> [128, b]) carry bias
     rows into hT for free
  4. group PAIRS share one [128,1024] bf16 PSUM transpose tile (one
     bank) and one hT drain; one matmul per 4-triple group with
     stationary bigmix [118, 51] computes self term + adjacency mix +
     bias for 51 (j,dm') outputs at once (matmuls lag one pair behind
     their transposes so the PE never stalls on the drain)
  5. drain 4 groups per 2-bank PSUM tile into o_sb [128, 2944] slices;
     three bf16 stores per tile; host un-permutes + upcasts.
"""

import os
import sys

for _p in ("/opt/trn_rl_repo", "/root/.axon_site/_ro/trn_rl_repo"):
    if os.path.isdir(_p) and _p not in sys.path:
        sys.path.insert(0, _p)

import numpy as np

import concourse.bass as bass
import concourse.mybir as mybir
import concourse.tile as tile
from concourse import bacc
from concourse import bass_utils as _bu
from concourse.bass_utils import run_bass_kernel_spmd

B, J, FIN, FOUT = 16384, 17, 128, 128
N_CORES = 8
TB = 128              # batch rows per tile
NT3 = 43              # m-triples per tile (128 = 3*43 - 1; (42,2) is pad)
NG = 11               # groups of <=4 triples: 10 full + 1 of 3 triples
GW = [512] * 10 + [384]          # mix free width per group
CDIM = 128            # transpose block: c = h*64 + k*3 + dm (+pads)
MIXK = 118            # mix contraction rows (bias rides c=54:57)
OFREE = 6 * 512 - 128  # o_sb free size: 5 paired blocks + 384 tail = 2944
ROWS2 = 128            # o_sb partitions: group pair at rows 0:51 and 64:115
F32 = mybir.dt.float32
BF16 = mybir.dt.bfloat16

_prog_cache: dict[tuple, object] = {}


def _build_program(bs: int, repeat: int = 1):
    """Build the SPMD Bass program for a per-core batch shard of `bs` rows."""
    nt = bs // TB
    assert bs % (2 * TB) == 0, "bs must be a multiple of 256 (paired tiles)"
    np2 = nt // 2

    nc = bacc.Bacc("TRN2", target_bir_lowering=False, debug=False,
                   num_devices=N_CORES)

    xt = nc.declare_dram_parameter("xt", [J, FIN, bs], BF16, isOutput=False)
    wcat = nc.declare_dram_parameter("wcat", [FIN, J, 258], BF16,
                                     isOutput=False)
    bigmix = nc.declare_dram_parameter("bigmix", [MIXK, 51], BF16,
                                       isOutput=False)
    biash = nc.declare_dram_parameter("biash", [TB, NT3, 3], BF16,
                                      isOutput=False)
    ident = nc.declare_dram_parameter("ident", [128, 128], BF16,
                                      isOutput=False)
    outT = nc.declare_dram_parameter("outT", [nt, ROWS2, OFREE], BF16,
                                     isOutput=True)

    with tile.TileContext(nc) as tc:
        with (
            tc.tile_pool(name="const", bufs=1) as cpool,
            tc.tile_pool(name="x", bufs=3) as xpool,
            tc.tile_pool(name="h", bufs=2) as hpool,
            tc.tile_pool(name="hT", bufs=5) as hTpool,
            tc.tile_pool(name="o", bufs=3) as opool,
            tc.tile_pool(name="hk", bufs=2, space=bass.MemorySpace.PSUM) as hkp,
            tc.tile_pool(name="tp", bufs=2, space=bass.MemorySpace.PSUM) as tpp,
            tc.tile_pool(name="mx", bufs=1, space=bass.MemorySpace.PSUM) as mxp,
        ):
            # ---- constants, loaded once ----
            wcat_sb = cpool.tile([FIN, J, 258], BF16, tag="wcat")
            nc.sync.dma_start(wcat_sb[:], wcat[:])
            bigmix_sb = cpool.tile([MIXK, 51], BF16, tag="bigmix")
            nc.sync.dma_start(bigmix_sb[:], bigmix[:])
            id_sb = cpool.tile([128, 128], BF16, tag="ident")
            nc.sync.dma_start(id_sb[:], ident[:])

            # engine-aware PSUM drains (gpsimd can't see PSUM):
            # bf16->bf16 hT drains run 2x on DVE; f32-source drains are
            # 1x everywhere, so spread them to balance busy time.
            def drain(dst, src, kind):
                if kind in ("hT", "hD"):
                    eng = nc.vector.tensor_copy
                else:  # "o" / "hA"
                    eng = nc.scalar.copy
                eng(dst, src)

            def stage1(x_t, half):
                # h_sb[b, t, h, c3], c3 = k*3 + dm; c3 = 51:54 (k=17
                # slot) and 57:64 / h=1 tail stay zero, c3 = 54:57 of
                # h=0 holds bias (both written once below).
                # Joints are processed in pairs: both matmuls of a pair
                # land in one 2-bank PSUM tile and drain in ONE copy.
                h_sb = hpool.tile([TB, NT3, 2, 64], BF16, tag="h")
                for p in range(9):
                    ks = [k for k in (2 * p, 2 * p + 1) if k < J]
                    hk = hkp.tile([TB, 2, 512], F32, tag="hk")
                    for k2, k in enumerate(ks):
                        nc.tensor.matmul(
                            hk[:, k2, 0:258],
                            x_t[:, k, half * TB:(half + 1) * TB],
                            wcat_sb[:, k, :])
                    if len(ks) == 2:
                        dst = h_sb[:, :, :, 6 * p:6 * p + 6].rearrange(
                            "b t h (k2 dm) -> b k2 t h dm", k2=2)
                        drain(dst, hk[:, :, 0:258],
                              "hA" if p in (0, 2, 4, 6, 7) else "hD")
                    else:
                        drain(h_sb[:, :, :, 6 * p:6 * p + 3],
                              hk[:, 0, 0:258], "hD")
                return h_sb

            def mix(h_sb, t_out):
                # groups are processed in PAIRS sharing one [128,1024]
                # bf16 PSUM tile (still one bank) -> half the hT drains;
                # mix matmuls lag one pair behind their transposes so
                # the PE chews transposes while the drain completes.
                o_sb = opool.tile([ROWS2, OFREE], BF16, tag="o")
                mps = {}
                pend = []

                def flush(g, hT2, off, w):
                    g4 = g // 4
                    if g % 4 == 0:
                        mps[g4] = mxp.tile([ROWS2, 1024], F32, tag="mx",
                                           name="mp")
                    sub, c0 = g % 2, ((g // 2) % 2) * 512
                    nc.tensor.matmul(
                        mps[g4][sub * 64:sub * 64 + 51, c0:c0 + w],
                        bigmix_sb[:], hT2[0:MIXK, off:off + w])
                    if g % 4 == 3 or g == NG - 1:
                        bw = c0 + w
                        drain(o_sb[:, g4 * 1024:g4 * 1024 + bw],
                              mps[g4][:, :bw], "o")
                        nc.sync.dma_start(
                            outT[t_out, :, g4 * 1024:g4 * 1024 + bw],
                            o_sb[:, g4 * 1024:g4 * 1024 + bw])

                for gp in range(6):
                    gs = [g for g in (2 * gp, 2 * gp + 1) if g < NG]
                    tot = sum(GW[g] for g in gs)
                    tp2 = tpp.tile([CDIM, 1024], BF16, tag="tp",
                                   name="tp2")
                    for i, g in enumerate(gs):
                        for ts in range(GW[g] // TB):
                            nc.tensor.transpose(
                                tp2[:, i * 512 + ts * TB:
                                    i * 512 + (ts + 1) * TB],
                                h_sb[:, 4 * g + ts, :, :], id_sb[:])
                    hT2 = hTpool.tile([CDIM, 1024], BF16, tag="hT",
                                      name="hT2")
                    drain(hT2[:, :tot], tp2[:, :tot], "hT")
                    for a in pend:
                        flush(*a)
                    pend = [(g, hT2, i * 512, GW[g])
                            for i, g in enumerate(gs)]
                for a in pend:
                    flush(*a)

            # one-time h-buffer init: zero pads (0*NaN = NaN in the mix
            # matmul otherwise) and plant the bias columns at h=0 c3=54:57
            for _ in range(2):
                hz = hpool.tile([TB, NT3, 2, 64], BF16, tag="h")
                nc.gpsimd.memset(hz[:], 0.0)
                nc.sync.dma_start(hz[:, :, 0, 54:57], biash[:])

            # software pipeline: stage1(t) runs on PE while the copy
            # engines drain t's h; mix(t-1) fills the PE meanwhile. The
            # next pair's x-load is emitted before this iteration's
            # stores so SP-queue ordering never delays the prefetch.
            def load_x(p):
                x_t = xpool.tile([FIN, J, 2 * TB], BF16, tag="x",
                                 name="x_t")
                nc.sync.dma_start(
                    x_t[:],
                    xt[:, :, p * 2 * TB:(p + 1) * 2 * TB]
                    .rearrange("j n b -> n j b"))
                return x_t

            prev = None
            total = np2 * repeat
            x_cur = load_x(0)
            for it in range(total):
                x_this = x_cur
                for half in (0, 1):
                    h_new = stage1(x_this, half)
                    if half == 0 and it + 1 < total:
                        x_cur = load_x((it + 1) % np2)
                    if prev is not None:
                        mix(*prev)
                    prev = (h_new, 2 * (it % np2) + half)
            if prev is not None:
                mix(*prev)

    nc.compile()
    return nc


def _host_prep(x, W, bias, adj, bs):
    """Build the per-core input maps (weights-only compute + layout)."""
    import ml_dtypes
    diag = np.diagonal(adj).astype(np.float32)
    off = (adj * (1.0 - np.eye(J, dtype=adj.dtype))).astype(np.float32)

    # stage-1 weights [FIN, J, 258]: col q = t*6 + h*3 + dm holds
    # (h==0 ? diag_k*W0_k : W1_k)[:, 3t+dm], zero at the m=128 pad
    wh = np.zeros((J, FIN, 2, 129), np.float32)
    wh[:, :, 0, :128] = diag[:, None, None] * W[0]
    wh[:, :, 1, :128] = W[1]
    wc = wh.reshape(J, FIN, 2, 43, 3).transpose(0, 1, 3, 2, 4)
    wcat = np.ascontiguousarray(wc.reshape(J, FIN, 258).transpose(1, 0, 2))

    # mix stationary [118, 51]: rows c = h*64 + k*3 + dm (pads zero),
    # rows 54:57 = bias pass-through; cols p = j*3 + dm'
    bm = np.zeros((MIXK, 51), np.float32)
    bm[np.arange(51), np.arange(51)] = 1.0          # h0 self rows
    for dm in range(3):
        for k in range(J):
            bm[64 + 3 * k + dm, dm::3] = off[:, k]  # h1 mix rows
        bm[54 + dm, dm::3] = 1.0                    # bias rows
    # bias plane [TB, 43, 3]: biash[b, t, dm] = bias[3t+dm] (b-bcast)
    mvals = 3 * np.arange(NT3)[:, None] + np.arange(3)[None, :]
    bvals = np.where(mvals < FOUT, bias[np.minimum(mvals, FOUT - 1)], 0.0)
    biash = np.ascontiguousarray(
        np.broadcast_to(bvals[None], (TB, NT3, 3))).astype(np.float32)

    shared = {
        "wcat": wcat.astype(ml_dtypes.bfloat16),
        "bigmix": bm.astype(ml_dtypes.bfloat16),
        "biash": biash.astype(ml_dtypes.bfloat16),
        "ident": np.eye(128, dtype=np.float32).astype(ml_dtypes.bfloat16),
    }
    in_maps = []
    for c in range(N_CORES):
        m = dict(shared)
        xs = x[c * bs:(c + 1) * bs]                  # [bs, J, FIN]
        m["xt"] = np.ascontiguousarray(
            xs.transpose(1, 2, 0)).astype(ml_dtypes.bfloat16)
        in_maps.append(m)
    return in_maps


_decode_idx_cache: dict[int, np.ndarray] = {}


def _decode_idx():
    """Flat gather indices: out[b,j,m] = outT_flat[tile, idx[j,m] + b]."""
    if 0 not in _decode_idx_cache:
        idx = np.zeros((J, FOUT), np.int64)
        for m in range(FOUT):
            t, dm = divmod(m, 3)
            g, ts = divmod(t, 4)
            for j in range(J):
                if g < 10:
                    row = (g % 2) * 64 + j * 3 + dm
                    col = (g // 2) * 512 + ts * TB
                else:
                    row = j * 3 + dm
                    col = 2560 + ts * TB
                idx[j, m] = row * OFREE + col
        _decode_idx_cache[0] = idx
    return _decode_idx_cache[0]


def _decode_out(outT_core, bs):
    """[nt, 102, OFREE] bf16 -> [bs, J, FOUT] f32."""
    nt = bs // TB
    flat = np.ascontiguousarray(outT_core).reshape(nt, ROWS2 * OFREE)
    idx = _decode_idx()                      # [J, FOUT]
    gather = flat[:, idx[None, :, :, None] +
                  np.arange(TB)[None, None, None, :]]  # [nt,1? J,FOUT,TB]
    gather = gather.reshape(nt, J, FOUT, TB)
    return np.ascontiguousarray(
        gather.transpose(0, 3, 1, 2)).reshape(bs, J, FOUT).astype(np.float32)


def _run(x, W, bias, adj, bs, profile=False, tmpdir=None):
    key = (bs,)
    if key not in _prog_cache:
        _prog_cache[key] = _build_program(bs)
    nc = _prog_cache[key]
    in_maps = _host_prep(x, W, bias, adj, bs)
    res = run_bass_kernel_spmd(nc, in_maps, list(range(N_CORES)),
                               trace=profile, tmpdir=tmpdir)
    out = np.concatenate(
        [_decode_out(res.results[c]["outT"], bs) for c in range(N_CORES)],
        axis=0)
    if profile:
        return out, res
    return out


def kernel(x, W, bias, adj):
    x = np.asarray(x, dtype=np.float32)
    W = np.asarray(W, dtype=np.float32)
    bias = np.asarray(bias, dtype=np.float32)
    adj = np.asarray(adj, dtype=np.float32)
    assert x.shape == (B, J, FIN)
    return _run(x, W, bias, adj, B // N_CORES)
